# revision 23
# baseline (speedup 1.0000x reference)
"""Trainium2 Bass kernel for AlphaFold-style pair attention (nn_Attention_90211493085692).

Reference computation (per batch b=1):
    q = (q_x @ w_q.T) / sqrt(C)         -> [N, Q, H, C]
    k = kv_x @ w_k.T ; v = kv_x @ w_v.T
    a = softmax(q @ k.T + mask_bias + pair_bias)   (softmax over k)
    o = (a @ v) * sigmoid(q_x @ w_g.T)
    out = o @ w_o.T

Sharding: outer pair dim N=256 is split across 8 cores (32 rows each);
weights / pair_bias replicated, each core computes its slab independently.

Device-side layout strategy (per core, ROWS = 32*256 = 8192 token rows):
  - host pre-transposes q_x/kv_x slabs to [CQ=128, ROWS] (feature-major) and
    casts to bf16, so every matmul contraction dim sits on SBUF partitions.
  - projections q^/k^/g^ in "transposed" layout [hc=128, rows] (stationary
    weight, moving input);  v in natural layout [rows, hc] (stationary input
    tile, moving weight).
  - per (n, ktile): scoresT[k, q] = k^T_h.T @ q^_h  (K=C=32 contraction),
    pair_bias handled as exp(s + B) = exp(s) * exp(B): exp on ACT, multiply by
    precomputed exp(pair_bias) on DVE.
  - sums over k via col-packed ones-matmul; attn@V via col-packed matmuls with
    v-natural stationary -> oT [hc, q]; normalize+gate on DVE; final W_O with
    gated oT stationary; DVE copy PSUM->SBUF; DMA out.
  - ROW-PIPELINED EMISSION: per row n the tensor queue gets
    [scores(n), sums/oV(n-1), W_O(n-2)] so the PE always has ready work while
    ACT/DVE chew on exp/normalize of the previous rows.
"""

import os
import sys

sys.path.insert(0, "/opt/trn_rl_repo")
sys.path.insert(0, "/opt/pypackages")

from contextlib import ExitStack

import ml_dtypes
import numpy as np

import concourse.bass as bass
import concourse.bacc as bacc
import concourse.tile as tile
from concourse import mybir

H = 4
C = 32
CQ = 128
N = 256
B = 1
NCORES = 8
NLOC = N // NCORES          # 32 outer rows per core
ROWS = NLOC * N             # 8192 token rows per core
P = 128

F32 = mybir.dt.float32
BF16 = mybir.dt.bfloat16
NP_BF16 = ml_dtypes.bfloat16
AF = mybir.ActivationFunctionType


def build_nc(use_mask: bool = True) -> bass.Bass:
    nc = bacc.Bacc()

    q_xt = nc.declare_dram_parameter("q_xt", [CQ, ROWS], BF16, isOutput=False)
    kv_xt = nc.declare_dram_parameter("kv_xt", [CQ, ROWS], BF16, isOutput=False)
    pbias = nc.declare_dram_parameter("pbias", [2, P, H, N], BF16, isOutput=False)
    ident = nc.declare_dram_parameter("ident", [P, P], BF16, isOutput=False)
    maskt = nc.declare_dram_parameter("maskt", [2, P, NLOC], F32, isOutput=False)
    w_qt = nc.declare_dram_parameter("w_qt", [CQ, P], BF16, isOutput=False)
    w_kt = nc.declare_dram_parameter("w_kt", [CQ, P], BF16, isOutput=False)
    w_vt = nc.declare_dram_parameter("w_vt", [CQ, P], BF16, isOutput=False)
    w_gt = nc.declare_dram_parameter("w_gt", [CQ, P], BF16, isOutput=False)
    w_ot = nc.declare_dram_parameter("w_ot", [P, CQ], BF16, isOutput=False)
    outt = nc.declare_dram_parameter("outt", [CQ, ROWS], BF16, isOutput=True)

    with tile.TileContext(nc) as tc, ExitStack() as ctx:
        const = ctx.enter_context(tc.tile_pool(name="const", bufs=1))

        NCHUNK = 1
        CROWS = ROWS // NCHUNK
        NSUB = 8
        SROWS = CROWS // NSUB          # 1024 rows per load subtile
        qx_sb = [[const.tile([P, SROWS], BF16, name=f"qx_sb{i}_{s}")
                  for s in range(NSUB)] for i in range(NCHUNK)]
        kx_sb = [[const.tile([P, SROWS], BF16, name=f"kx_sb{i}_{s}")
                  for s in range(NSUB)] for i in range(NCHUNK)]
        for i in range(NCHUNK):
            for s in range(NSUB):
                off = i * CROWS + s * SROWS
                # split the two input streams across independent DMA
                # queues (sync + gpsimd) so they transfer in parallel
                nc.sync.dma_start(out=qx_sb[i][s][:],
                                  in_=q_xt[:, off:off + SROWS])
                nc.gpsimd.dma_start(out=kx_sb[i][s][:],
                                     in_=kv_xt[:, off:off + SROWS])
        wq_sb = const.tile([CQ, P], BF16)
        nc.sync.dma_start(out=wq_sb[:], in_=w_qt[:])
        wk_sb = const.tile([CQ, P], BF16)
        nc.sync.dma_start(out=wk_sb[:], in_=w_kt[:])
        wv_sb = const.tile([CQ, P], BF16)
        nc.sync.dma_start(out=wv_sb[:], in_=w_vt[:])
        wg_sb = const.tile([CQ, P], BF16)
        nc.sync.dma_start(out=wg_sb[:], in_=w_gt[:])
        wo_sb = const.tile([P, CQ], BF16)
        nc.sync.dma_start(out=wo_sb[:], in_=w_ot[:])
        id_sb = const.tile([P, P], BF16)
        nc.sync.dma_start(out=id_sb[:], in_=ident[:])

        mask_sb = const.tile([P, 2, NLOC], F32)
        nc.sync.dma_start(out=mask_sb[:], in_=maskt.rearrange("t p n -> p t n"))

        pb_sb = const.tile([P, 2, H, N], BF16)
        nc.sync.dma_start(out=pb_sb[:], in_=pbias.rearrange("t p h q -> p t h q"))

        # 2.0 (not 1.0): bakes the 0.5 of sigmoid(x)=0.5*(1+tanh(x/2)) into
        # the softmax denominator so inv = 0.5/sum
        ones_sb = const.tile([P, C], BF16)
        nc.vector.memset(ones_sb[:], 2.0)

        qhat = [const.tile([P, CROWS], BF16, name=f"qhat{i}")
                for i in range(NCHUNK)]
        khat = [const.tile([P, CROWS], BF16, name=f"khat{i}")
                for i in range(NCHUNK)]
        ghat = [const.tile([P, CROWS], BF16, name=f"ghat{i}")
                for i in range(NCHUNK)]
        vnat = [const.tile([P, CROWS // P, P], BF16, name=f"vnat{i}")
                for i in range(NCHUNK)]

        # ------- interleaved: projections chunk i, then attention rows -------
        CHUNK = 512
        with tc.tile_pool(name="scps", bufs=2, space="PSUM") as sp, \
             tc.tile_pool(name="accps", bufs=4, space="PSUM") as acc, \
             tc.tile_pool(name="work", bufs=4) as wrk, \
             tc.tile_pool(name="expp", bufs=6) as expp:

            def proj_q(j):
                s, ch = divmod(j, SROWS // CHUNK)
                sl = slice(ch * CHUNK, (ch + 1) * CHUNK)
                gsl = slice(j * CHUNK, (j + 1) * CHUNK)
                ps_q = acc.tile([P, CHUNK], F32, tag="acc", name=f"ps_q{j}")
                nc.tensor.matmul(ps_q[:], lhsT=wq_sb[:], rhs=qx_sb[0][s][:, sl],
                                 start=True, stop=True)
                nc.vector.tensor_copy(qhat[0][:, gsl], ps_q[:])

            def proj_k(j):
                s, ch = divmod(j, SROWS // CHUNK)
                sl = slice(ch * CHUNK, (ch + 1) * CHUNK)
                gsl = slice(j * CHUNK, (j + 1) * CHUNK)
                ps_k = acc.tile([P, CHUNK], F32, tag="acc", name=f"ps_k{j}")
                nc.tensor.matmul(ps_k[:], lhsT=wk_sb[:], rhs=kx_sb[0][s][:, sl],
                                 start=True, stop=True)
                nc.vector.tensor_copy(khat[0][:, gsl], ps_k[:])

            def proj_g(j):
                s, ch = divmod(j, SROWS // CHUNK)
                sl = slice(ch * CHUNK, (ch + 1) * CHUNK)
                gsl = slice(j * CHUNK, (j + 1) * CHUNK)
                ps_g = acc.tile([P, CHUNK], F32, tag="acc", name=f"ps_g{j}")
                nc.tensor.matmul(ps_g[:], lhsT=wg_sb[:], rhs=qx_sb[0][s][:, sl],
                                 start=True, stop=True)
                # sigmoid(x)=0.5*(1+tanh(x/2)); same ACT table as exp
                nc.scalar.activation(out=ghat[0][:, gsl], in_=ps_g[:],
                                     func=AF.Tanh, scale=0.5)

            def proj_v(j):
                s, ch = divmod(j, SROWS // CHUNK)
                sl = slice(ch * CHUNK, (ch + 1) * CHUNK)
                ps_v = acc.tile([P, 4, P], F32, tag="acc", name=f"ps_v{j}")
                for jj in range(4):
                    nc.tensor.matmul(
                        ps_v[:, jj, :],
                        lhsT=qx_sb[0][s][:, sl][:, jj * P:(jj + 1) * P] if False else kx_sb[0][s][:, sl][:, jj * P:(jj + 1) * P],
                        rhs=wv_sb[:],
                        start=(jj == 0), stop=(jj == 3))
                goff = j * CHUNK // P
                nc.vector.tensor_copy(vnat[0][:, goff:goff + 4, :], ps_v[:])

            expT = {}

            def emit_scores(n):
                q0 = n * N
                cidx = q0 // CROWS
                coff = cidx * CROWS
                et = expp.tile([P, 2, H, N], BF16, tag="expT",
                               name=f"expT{n}")
                expT[n] = et
                for t in range(2):
                    k0 = q0 + t * P
                    for hp in range(2):
                        # concurrent (tile-packed) score matmuls must write
                        # DISTINCT PSUM banks: each scp tile provides one
                        # bank per head of the hp-pair.
                        scp = sp.tile([P, 2, 512], F32, tag="sc",
                                      name=f"sc_{n}_{t}_{hp}")
                        for hi in range(2):
                            h = 2 * hp + hi
                            nc.tensor.matmul(
                                scp[:, hi, :N],
                                lhsT=khat[cidx][32 * h:32 * h + 32,
                                                k0 - coff:k0 - coff + P],
                                rhs=qhat[cidx][32 * h:32 * h + 32,
                                               q0 - coff:q0 - coff + N],
                                start=True, stop=False,
                                tile_position=(32 * h, 0),
                                skip_group_check=True,
                            )
                        # pair_bias accumulated by the PE (identity
                        # stationary, pb moving) instead of a DVE multiply
                        # by exp(pair_bias).
                        for hi in range(2):
                            h = 2 * hp + hi
                            nc.tensor.matmul(
                                scp[:, hi, :N],
                                lhsT=id_sb[:],
                                rhs=pb_sb[:, t, h, :],
                                start=False, stop=True,
                                skip_group_check=True,
                            )
                        # exp(s + pb + mask): mask rides the per-partition
                        # activation bias port.
                        if use_mask:
                            nc.scalar.activation(
                                out=et[:, t, 2 * hp:2 * hp + 2, :],
                                in_=scp[:, :, :N], func=AF.Exp,
                                bias=mask_sb[:, t, n:n + 1])
                        else:
                            nc.scalar.activation(
                                out=et[:, t, 2 * hp:2 * hp + 2, :],
                                in_=scp[:, :, :N], func=AF.Exp)

            def emit_gproj(s):
                for ch in range(SROWS // CHUNK):
                    sl = slice(s * SROWS + ch * CHUNK,
                               s * SROWS + (ch + 1) * CHUNK)
                    ps_g = acc.tile([P, CHUNK], F32, tag="acc",
                                    name=f"ps_g{s}_{ch}")
                    nc.tensor.matmul(ps_g[:], lhsT=wg_sb[:],
                                     rhs=qx_sb[0][s][:, sl - 0 if False else slice(ch * CHUNK, (ch + 1) * CHUNK)],
                                     start=True, stop=True)
                    # sigmoid(x)=0.5*(1+tanh(x/2)); same ACT table as exp
                    nc.scalar.activation(out=ghat[0][:, sl], in_=ps_g[:],
                                         func=AF.Tanh, scale=0.5)

            gated = {}

            def emit_ovsums(n):
                q0 = n * N
                cidx = q0 // CROWS
                coff = cidx * CROWS
                et = expT.pop(n)
                sums_ps = acc.tile([P, N], F32, tag="acc", name=f"sums_ps{n}")
                o_ps = acc.tile([P, N], F32, tag="acc", name=f"o_ps{n}")
                for t in range(2):
                    for h in range(H):
                        nc.tensor.matmul(
                            sums_ps[32 * h:32 * h + 32, :],
                            lhsT=ones_sb[:],
                            rhs=et[:, t, h, :],
                            start=(t == 0), stop=(t == 1),
                            tile_position=(0, 32 * h),
                            skip_group_check=True,
                        )
                    for h in range(H):
                        nc.tensor.matmul(
                            o_ps[32 * h:32 * h + 32, :],
                            lhsT=vnat[cidx][:, (q0 - coff) // P + t,
                                            32 * h:32 * h + 32],
                            rhs=et[:, t, h, :],
                            start=(t == 0), stop=(t == 1),
                            tile_position=(0, 32 * h),
                            skip_group_check=True,
                        )

                inv = wrk.tile([P, N], F32, tag="inv", name=f"inv{n}")
                nc.vector.reciprocal_approx_fast(out=inv[:], in_=sums_ps[:])
                onrm = wrk.tile([P, N], BF16, tag="onrm", name=f"onrm{n}")
                nc.vector.tensor_mul(out=onrm[:], in0=o_ps[:], in1=inv[:])
                pair = n // 2
                if n % 2 == 0:
                    gated[pair] = wrk.tile([P, 2, N], BF16, tag="gated",
                                           name=f"gated{pair}")
                nc.vector.scalar_tensor_tensor(
                    out=gated[pair][:, n % 2, :],
                    in0=ghat[cidx][:, q0 - coff:q0 - coff + N],
                    scalar=1.0,
                    in1=onrm[:],
                    op0=mybir.AluOpType.add,
                    op1=mybir.AluOpType.mult)

            def emit_wo(pair):
                n0 = 2 * pair
                gt = gated.pop(pair)
                wo_ps = acc.tile([P, 2 * N], F32, tag="acc", name=f"wo_ps{pair}")
                nc.tensor.matmul(wo_ps[:], lhsT=wo_sb[:], rhs=gt[:],
                                 start=True, stop=True)
                osb = wrk.tile([P, 2 * N], BF16, tag="osb", name=f"osb{pair}")
                nc.vector.tensor_copy(osb[:], wo_ps[:])
                nc.sync.dma_start(out=outt[:, n0 * N:(n0 + 2) * N],
                                  in_=osb[:])

            NSLICE = ROWS // CHUNK
            for j in range(2):
                proj_q(j); proj_k(j); proj_g(j); proj_v(j)
            for n in range(NLOC):
                j = n + 2
                if j < NSLICE:
                    proj_q(j)
                    proj_k(j)
                emit_scores(n)
                if n - 1 >= 0:
                    emit_ovsums(n - 1)
                    if (n - 1) % 2 == 1:
                        emit_wo((n - 1) // 2)
                if j < NSLICE:
                    proj_g(j)
                    proj_v(j)
            emit_ovsums(NLOC - 1)
            emit_wo((NLOC - 1) // 2)

    nc.compile()
    return nc


_CACHE: dict = {}


def _get_nc(use_mask: bool = False) -> bass.Bass:
    key = ("nc", use_mask)
    if key not in _CACHE:
        _CACHE[key] = build_nc(use_mask=use_mask)
    return _CACHE[key]


def make_in_maps(q_x, kv_x, mask_bias, pair_bias, w_q, w_k, w_v, w_g, w_o):
    qf = np.asarray(q_x, dtype=np.float32).reshape(NCORES, ROWS, CQ)
    kf = np.asarray(kv_x, dtype=np.float32).reshape(NCORES, ROWS, CQ)
    mb = np.asarray(mask_bias, dtype=np.float32).reshape(N, N)      # [n, k]
    pb = np.asarray(pair_bias, dtype=np.float32).reshape(H, N, N)   # [h, q, k]

    pbT = np.transpose(pb, (2, 0, 1))                 # [k, h, q]
    pb_dev = np.ascontiguousarray(pbT.reshape(2, P, H, N)).astype(NP_BF16)

    wqt = np.ascontiguousarray((w_q / np.sqrt(C)).T).astype(NP_BF16)
    wkt = np.ascontiguousarray(np.asarray(w_k).T).astype(NP_BF16)
    wvt = np.ascontiguousarray(np.asarray(w_v).T).astype(NP_BF16)
    wgt = np.ascontiguousarray(np.asarray(w_g).T).astype(NP_BF16)
    wot = np.ascontiguousarray(np.asarray(w_o).T).astype(NP_BF16)   # [hc, cq]

    in_maps = []
    for c in range(NCORES):
        m = mb[c * NLOC:(c + 1) * NLOC]               # [nloc, k]
        mT = np.ascontiguousarray(
            np.transpose(m.reshape(NLOC, 2, P), (1, 2, 0))).astype(np.float32)
        in_maps.append({
            "q_xt": np.ascontiguousarray(qf[c].T).astype(NP_BF16),
            "kv_xt": np.ascontiguousarray(kf[c].T).astype(NP_BF16),
            "pbias": pb_dev,
            "ident": np.eye(P, dtype=NP_BF16),
            "maskt": mT,
            "w_qt": wqt, "w_kt": wkt, "w_vt": wvt, "w_gt": wgt, "w_ot": wot,
        })
    return in_maps


def gather_out(res):
    outs = []
    for c in range(NCORES):
        ot = np.asarray(res.results[c]["outt"])       # [CQ, ROWS] bf16
        outs.append(ot.astype(np.float32).T)          # [ROWS, CQ]
    return np.concatenate(outs, axis=0).reshape(B, N, N, CQ)


def kernel(q_x, kv_x, mask_bias, pair_bias, w_q, w_k, w_v, w_g, w_o):
    from concourse.bass_utils import run_bass_kernel_spmd

    nc = _get_nc(use_mask=bool(np.any(np.asarray(mask_bias))))
    in_maps = make_in_maps(q_x, kv_x, mask_bias, pair_bias,
                           w_q, w_k, w_v, w_g, w_o)
    res = run_bass_kernel_spmd(nc, in_maps, list(range(NCORES)))
    return gather_out(res)



# revision 24
# speedup vs baseline: 1.1847x; 1.1847x over previous
"""Trainium2 Bass kernel for AlphaFold-style pair attention (nn_Attention_90211493085692).

Reference computation (per batch b=1):
    q = (q_x @ w_q.T) / sqrt(C)         -> [N, Q, H, C]
    k = kv_x @ w_k.T ; v = kv_x @ w_v.T
    a = softmax(q @ k.T + mask_bias + pair_bias)   (softmax over k)
    o = (a @ v) * sigmoid(q_x @ w_g.T)
    out = o @ w_o.T

Sharding: outer pair dim N=256 is split across 8 cores (32 rows each);
weights / pair_bias replicated, each core computes its slab independently.

Device-side layout strategy (per core, ROWS = 32*256 = 8192 token rows):
  - host pre-transposes q_x/kv_x slabs to [CQ=128, ROWS] (feature-major) and
    casts to bf16, so every matmul contraction dim sits on SBUF partitions.
  - projections q^/k^/g^ in "transposed" layout [hc=128, rows] (stationary
    weight, moving input);  v in natural layout [rows, hc] (stationary input
    tile, moving weight).
  - per (n, ktile): scoresT[k, q] = k^T_h.T @ q^_h  (K=C=32 contraction),
    pair_bias handled as exp(s + B) = exp(s) * exp(B): exp on ACT, multiply by
    precomputed exp(pair_bias) on DVE.
  - sums over k via col-packed ones-matmul; attn@V via col-packed matmuls with
    v-natural stationary -> oT [hc, q]; normalize+gate on DVE; final W_O with
    gated oT stationary; DVE copy PSUM->SBUF; DMA out.
  - ROW-PIPELINED EMISSION: per row n the tensor queue gets
    [scores(n), sums/oV(n-1), W_O(n-2)] so the PE always has ready work while
    ACT/DVE chew on exp/normalize of the previous rows.
"""

import os
import sys

sys.path.insert(0, "/opt/trn_rl_repo")
sys.path.insert(0, "/opt/pypackages")

from contextlib import ExitStack

import ml_dtypes
import numpy as np

import concourse.bass as bass
import concourse.bacc as bacc
import concourse.tile as tile
from concourse import mybir

H = 4
C = 32
CQ = 128
N = 256
B = 1
NCORES = 8
NLOC = N // NCORES          # 32 outer rows per core
ROWS = NLOC * N             # 8192 token rows per core
P = 128

F32 = mybir.dt.float32
BF16 = mybir.dt.bfloat16
NP_BF16 = ml_dtypes.bfloat16
AF = mybir.ActivationFunctionType


def build_nc(use_mask: bool = True) -> bass.Bass:
    nc = bacc.Bacc()

    q_xt = nc.declare_dram_parameter("q_xt", [CQ, ROWS], BF16, isOutput=False)
    kv_xt = nc.declare_dram_parameter("kv_xt", [CQ, ROWS], BF16, isOutput=False)
    pbias = nc.declare_dram_parameter("pbias", [2, P, H, N], BF16, isOutput=False)
    ident = nc.declare_dram_parameter("ident", [P, P], BF16, isOutput=False)
    maskt = nc.declare_dram_parameter("maskt", [2, P, NLOC], F32, isOutput=False)
    w_qt = nc.declare_dram_parameter("w_qt", [CQ, P], BF16, isOutput=False)
    w_kt = nc.declare_dram_parameter("w_kt", [CQ, P], BF16, isOutput=False)
    w_vt = nc.declare_dram_parameter("w_vt", [CQ, P], BF16, isOutput=False)
    w_gt = nc.declare_dram_parameter("w_gt", [CQ, P], BF16, isOutput=False)
    w_ot = nc.declare_dram_parameter("w_ot", [P, CQ], BF16, isOutput=False)
    out = nc.declare_dram_parameter("out", [ROWS, CQ], F32, isOutput=True)

    with tile.TileContext(nc) as tc, ExitStack() as ctx:
        const = ctx.enter_context(tc.tile_pool(name="const", bufs=1))

        NCHUNK = 1
        CROWS = ROWS // NCHUNK
        NSUB = 8
        SROWS = CROWS // NSUB          # 1024 rows per load subtile
        qx_sb = [[const.tile([P, SROWS], BF16, name=f"qx_sb{i}_{s}")
                  for s in range(NSUB)] for i in range(NCHUNK)]
        kx_sb = [[const.tile([P, SROWS], BF16, name=f"kx_sb{i}_{s}")
                  for s in range(NSUB)] for i in range(NCHUNK)]
        for i in range(NCHUNK):
            for s in range(NSUB):
                off = i * CROWS + s * SROWS
                # split the two input streams across independent DMA
                # queues (sync + gpsimd) so they transfer in parallel
                nc.sync.dma_start(out=qx_sb[i][s][:],
                                  in_=q_xt[:, off:off + SROWS])
                nc.gpsimd.dma_start(out=kx_sb[i][s][:],
                                     in_=kv_xt[:, off:off + SROWS])
        wq_sb = const.tile([CQ, P], BF16)
        nc.sync.dma_start(out=wq_sb[:], in_=w_qt[:])
        wk_sb = const.tile([CQ, P], BF16)
        nc.sync.dma_start(out=wk_sb[:], in_=w_kt[:])
        wv_sb = const.tile([CQ, P], BF16)
        nc.sync.dma_start(out=wv_sb[:], in_=w_vt[:])
        wg_sb = const.tile([CQ, P], BF16)
        nc.sync.dma_start(out=wg_sb[:], in_=w_gt[:])
        wo_sb = const.tile([P, CQ], BF16)
        nc.sync.dma_start(out=wo_sb[:], in_=w_ot[:])
        id_sb = const.tile([P, P], BF16)
        nc.sync.dma_start(out=id_sb[:], in_=ident[:])

        mask_sb = const.tile([P, 2, NLOC], F32)
        nc.sync.dma_start(out=mask_sb[:], in_=maskt.rearrange("t p n -> p t n"))

        pb_sb = const.tile([P, 2, H, N], BF16)
        nc.sync.dma_start(out=pb_sb[:], in_=pbias.rearrange("t p h q -> p t h q"))

        # 2.0 (not 1.0): bakes the 0.5 of sigmoid(x)=0.5*(1+tanh(x/2)) into
        # the softmax denominator so inv = 0.5/sum
        ones_sb = const.tile([P, C], BF16)
        nc.vector.memset(ones_sb[:], 2.0)

        qhat = [const.tile([P, CROWS], BF16, name=f"qhat{i}")
                for i in range(NCHUNK)]
        khat = [const.tile([P, CROWS], BF16, name=f"khat{i}")
                for i in range(NCHUNK)]
        ghat = [const.tile([P, CROWS], BF16, name=f"ghat{i}")
                for i in range(NCHUNK)]
        vnat = [const.tile([P, CROWS // P, P], BF16, name=f"vnat{i}")
                for i in range(NCHUNK)]

        # ------- interleaved: projections chunk i, then attention rows -------
        CHUNK = 512
        with tc.tile_pool(name="scps", bufs=2, space="PSUM") as sp, \
             tc.tile_pool(name="accps", bufs=4, space="PSUM") as acc, \
             tc.tile_pool(name="work", bufs=4) as wrk, \
             tc.tile_pool(name="expp", bufs=6) as expp:

            def emit_proj_chunk(i):
                for s in range(NSUB):
                    for ch in range(SROWS // CHUNK):
                        sl = slice(ch * CHUNK, (ch + 1) * CHUNK)
                        gsl = slice(s * SROWS + ch * CHUNK,
                                    s * SROWS + (ch + 1) * CHUNK)
                        ps_q = acc.tile([P, CHUNK], F32, tag="acc",
                                        name=f"ps_q{i}_{s}_{ch}")
                        nc.tensor.matmul(ps_q[:], lhsT=wq_sb[:],
                                         rhs=qx_sb[i][s][:, sl],
                                         start=True, stop=True)
                        nc.vector.tensor_copy(qhat[i][:, gsl], ps_q[:])

                        ps_k = acc.tile([P, CHUNK], F32, tag="acc",
                                        name=f"ps_k{i}_{s}_{ch}")
                        nc.tensor.matmul(ps_k[:], lhsT=wk_sb[:],
                                         rhs=kx_sb[i][s][:, sl],
                                         start=True, stop=True)
                        nc.vector.tensor_copy(khat[i][:, gsl], ps_k[:])

                    for grp in range(SROWS // (4 * P)):
                        ps_v = acc.tile([P, 4, P], F32, tag="acc",
                                        name=f"ps_v{i}_{s}_{grp}")
                        for j in range(4):
                            rt = grp * 4 + j
                            nc.tensor.matmul(
                                ps_v[:, j, :],
                                lhsT=kx_sb[i][s][:, rt * P:(rt + 1) * P],
                                rhs=wv_sb[:],
                                start=(j == 0), stop=(j == 3))
                        goff = (s * SROWS) // P + grp * 4
                        # ACT-side copy: Copy shares the exp/tanh table set
                        nc.scalar.copy(vnat[i][:, goff:goff + 4, :], ps_v[:])

            expT = {}

            def emit_scores(n):
                q0 = n * N
                cidx = q0 // CROWS
                coff = cidx * CROWS
                et = expp.tile([P, 2, H, N], BF16, tag="expT",
                               name=f"expT{n}")
                expT[n] = et
                for t in range(2):
                    k0 = q0 + t * P
                    for hp in range(2):
                        scp = sp.tile([P, 2, 512], F32, tag="sc",
                                      name=f"sc_{n}_{t}_{hp}")
                        for hi in range(2):
                            h = 2 * hp + hi
                            nc.tensor.matmul(
                                scp[:, hi, :N],
                                lhsT=khat[cidx][32 * h:32 * h + 32,
                                                k0 - coff:k0 - coff + P],
                                rhs=qhat[cidx][32 * h:32 * h + 32,
                                               q0 - coff:q0 - coff + N],
                                start=True, stop=False,
                                tile_position=(32 * h, 0),
                                skip_group_check=True,
                            )
                        # pair_bias accumulated by the PE (identity
                        # stationary, pb moving) instead of a DVE multiply
                        # by exp(pair_bias).
                        for hi in range(2):
                            h = 2 * hp + hi
                            nc.tensor.matmul(
                                scp[:, hi, :N],
                                lhsT=id_sb[:],
                                rhs=pb_sb[:, t, h, :],
                                start=False, stop=True,
                                skip_group_check=True,
                            )
                        # exp(s + pb + mask): mask rides the per-partition
                        # activation bias port.
                        if use_mask:
                            nc.scalar.activation(
                                out=et[:, t, 2 * hp:2 * hp + 2, :],
                                in_=scp[:, :, :N], func=AF.Exp,
                                bias=mask_sb[:, t, n:n + 1])
                        else:
                            nc.scalar.activation(
                                out=et[:, t, 2 * hp:2 * hp + 2, :],
                                in_=scp[:, :, :N], func=AF.Exp)

            def emit_gproj(s):
                for ch in range(SROWS // CHUNK):
                    sl = slice(s * SROWS + ch * CHUNK,
                               s * SROWS + (ch + 1) * CHUNK)
                    ps_g = acc.tile([P, CHUNK], F32, tag="acc",
                                    name=f"ps_g{s}_{ch}")
                    nc.tensor.matmul(ps_g[:], lhsT=wg_sb[:],
                                     rhs=qx_sb[0][s][:, sl - 0 if False else slice(ch * CHUNK, (ch + 1) * CHUNK)],
                                     start=True, stop=True)
                    # sigmoid(x)=0.5*(1+tanh(x/2)); same ACT table as exp
                    nc.scalar.activation(out=ghat[0][:, sl], in_=ps_g[:],
                                         func=AF.Tanh, scale=0.5)

            gated = {}

            def emit_ovsums(n):
                q0 = n * N
                cidx = q0 // CROWS
                coff = cidx * CROWS
                et = expT.pop(n)
                sums_ps = acc.tile([P, N], F32, tag="acc", name=f"sums_ps{n}")
                o_ps = acc.tile([P, N], F32, tag="acc", name=f"o_ps{n}")
                for t in range(2):
                    for h in range(H):
                        nc.tensor.matmul(
                            sums_ps[32 * h:32 * h + 32, :],
                            lhsT=ones_sb[:],
                            rhs=et[:, t, h, :],
                            start=(t == 0), stop=(t == 1),
                            tile_position=(0, 32 * h),
                            skip_group_check=True,
                        )
                    for h in range(H):
                        nc.tensor.matmul(
                            o_ps[32 * h:32 * h + 32, :],
                            lhsT=vnat[cidx][:, (q0 - coff) // P + t,
                                            32 * h:32 * h + 32],
                            rhs=et[:, t, h, :],
                            start=(t == 0), stop=(t == 1),
                            tile_position=(0, 32 * h),
                            skip_group_check=True,
                        )

                inv = wrk.tile([P, N], F32, tag="inv", name=f"inv{n}")
                nc.vector.reciprocal_approx_fast(out=inv[:], in_=sums_ps[:])
                onrm = wrk.tile([P, N], BF16, tag="onrm", name=f"onrm{n}")
                nc.vector.tensor_mul(out=onrm[:], in0=o_ps[:], in1=inv[:])
                gated[n] = wrk.tile([P, N], BF16, tag="gated", name=f"gated{n}")
                nc.vector.scalar_tensor_tensor(
                    out=gated[n][:],
                    in0=ghat[cidx][:, q0 - coff:q0 - coff + N],
                    scalar=1.0,
                    in1=onrm[:],
                    op0=mybir.AluOpType.add,
                    op1=mybir.AluOpType.mult)

            def emit_wo(n):
                q0 = n * N
                gt = gated.pop(n)
                wo_ps = acc.tile([P, 2, 256], F32, tag="acc", name=f"wo_ps{n}")
                for qt in range(2):
                    nc.tensor.matmul(wo_ps[:, qt, :CQ],
                                     lhsT=gt[:, qt * P:(qt + 1) * P],
                                     rhs=wo_sb[:],
                                     start=(qt == 0), stop=(qt == 1))
                osb = wrk.tile([P, 2, CQ], F32, tag="osb", name=f"osb{n}")
                nc.vector.tensor_copy(osb[:], wo_ps[:, :, :CQ])
                nc.sync.dma_start(
                    out=out[q0:q0 + N, :].rearrange("(t p) c -> p t c", p=P),
                    in_=osb[:])

            emit_proj_chunk(0)
            emit_gproj(0)
            for n in range(NLOC):
                emit_scores(n)
                if n - 1 >= 0:
                    emit_ovsums(n - 1)
                if n - 2 >= 0:
                    emit_wo(n - 2)
                if n >= 2 and (n + 2) % 4 == 0 and (n + 2) // 4 < NSUB:
                    emit_gproj((n + 2) // 4)
            emit_ovsums(NLOC - 1)
            emit_wo(NLOC - 2)
            emit_wo(NLOC - 1)

    nc.compile()
    return nc


_CACHE: dict = {}


def _get_nc(use_mask: bool = False) -> bass.Bass:
    key = ("nc", use_mask)
    if key not in _CACHE:
        _CACHE[key] = build_nc(use_mask=use_mask)
    return _CACHE[key]


def make_in_maps(q_x, kv_x, mask_bias, pair_bias, w_q, w_k, w_v, w_g, w_o):
    qf = np.asarray(q_x, dtype=np.float32).reshape(NCORES, ROWS, CQ)
    kf = np.asarray(kv_x, dtype=np.float32).reshape(NCORES, ROWS, CQ)
    mb = np.asarray(mask_bias, dtype=np.float32).reshape(N, N)      # [n, k]
    pb = np.asarray(pair_bias, dtype=np.float32).reshape(H, N, N)   # [h, q, k]

    pbT = np.transpose(pb, (2, 0, 1))                 # [k, h, q]
    pb_dev = np.ascontiguousarray(pbT.reshape(2, P, H, N)).astype(NP_BF16)

    wqt = np.ascontiguousarray((w_q / np.sqrt(C)).T).astype(NP_BF16)
    wkt = np.ascontiguousarray(np.asarray(w_k).T).astype(NP_BF16)
    wvt = np.ascontiguousarray(np.asarray(w_v).T).astype(NP_BF16)
    wgt = np.ascontiguousarray(np.asarray(w_g).T).astype(NP_BF16)
    wot = np.ascontiguousarray(np.asarray(w_o).T).astype(NP_BF16)   # [hc, cq]

    in_maps = []
    for c in range(NCORES):
        m = mb[c * NLOC:(c + 1) * NLOC]               # [nloc, k]
        mT = np.ascontiguousarray(
            np.transpose(m.reshape(NLOC, 2, P), (1, 2, 0))).astype(np.float32)
        in_maps.append({
            "q_xt": np.ascontiguousarray(qf[c].T).astype(NP_BF16),
            "kv_xt": np.ascontiguousarray(kf[c].T).astype(NP_BF16),
            "pbias": pb_dev,
            "ident": np.eye(P, dtype=NP_BF16),
            "maskt": mT,
            "w_qt": wqt, "w_kt": wkt, "w_vt": wvt, "w_gt": wgt, "w_ot": wot,
        })
    return in_maps


def gather_out(res):
    outs = [np.asarray(res.results[c]["out"], dtype=np.float32)
            for c in range(NCORES)]
    return np.concatenate(outs, axis=0).reshape(B, N, N, CQ)


def kernel(q_x, kv_x, mask_bias, pair_bias, w_q, w_k, w_v, w_g, w_o):
    from concourse.bass_utils import run_bass_kernel_spmd

    nc = _get_nc(use_mask=bool(np.any(np.asarray(mask_bias))))
    in_maps = make_in_maps(q_x, kv_x, mask_bias, pair_bias,
                           w_q, w_k, w_v, w_g, w_o)
    res = run_bass_kernel_spmd(nc, in_maps, list(range(NCORES)))
    return gather_out(res)



# revision 25
# speedup vs baseline: 1.2326x; 1.0404x over previous
"""Trainium2 Bass kernel for AlphaFold-style pair attention (nn_Attention_90211493085692).

Reference computation (per batch b=1):
    q = (q_x @ w_q.T) / sqrt(C)         -> [N, Q, H, C]
    k = kv_x @ w_k.T ; v = kv_x @ w_v.T
    a = softmax(q @ k.T + mask_bias + pair_bias)   (softmax over k)
    o = (a @ v) * sigmoid(q_x @ w_g.T)
    out = o @ w_o.T

Sharding: outer pair dim N=256 is split across 8 cores (32 rows each);
weights / pair_bias replicated, each core computes its slab independently.

Device-side layout strategy (per core, ROWS = 32*256 = 8192 token rows):
  - host pre-transposes q_x/kv_x slabs to [CQ=128, ROWS] (feature-major) and
    casts to bf16, so every matmul contraction dim sits on SBUF partitions.
  - projections q^/k^/g^ in "transposed" layout [hc=128, rows] (stationary
    weight, moving input);  v in natural layout [rows, hc] (stationary input
    tile, moving weight).
  - per (n, ktile): scoresT[k, q] = k^T_h.T @ q^_h  (K=C=32 contraction),
    pair_bias handled as exp(s + B) = exp(s) * exp(B): exp on ACT, multiply by
    precomputed exp(pair_bias) on DVE.
  - sums over k via col-packed ones-matmul; attn@V via col-packed matmuls with
    v-natural stationary -> oT [hc, q]; normalize+gate on DVE; final W_O with
    gated oT stationary; DVE copy PSUM->SBUF; DMA out.
  - ROW-PIPELINED EMISSION: per row n the tensor queue gets
    [scores(n), sums/oV(n-1), W_O(n-2)] so the PE always has ready work while
    ACT/DVE chew on exp/normalize of the previous rows.
"""

import os
import sys

sys.path.insert(0, "/opt/trn_rl_repo")
sys.path.insert(0, "/opt/pypackages")

from contextlib import ExitStack

import ml_dtypes
import numpy as np

import concourse.bass as bass
import concourse.bacc as bacc
import concourse.tile as tile
from concourse import mybir

H = 4
C = 32
CQ = 128
N = 256
B = 1
NCORES = 8
NLOC = N // NCORES          # 32 outer rows per core
ROWS = NLOC * N             # 8192 token rows per core
P = 128

F32 = mybir.dt.float32
BF16 = mybir.dt.bfloat16
NP_BF16 = ml_dtypes.bfloat16
AF = mybir.ActivationFunctionType


def build_nc(use_mask: bool = True) -> bass.Bass:
    nc = bacc.Bacc()

    q_xt = nc.declare_dram_parameter("q_xt", [CQ, ROWS], BF16, isOutput=False)
    kv_xt = nc.declare_dram_parameter("kv_xt", [CQ, ROWS], BF16, isOutput=False)
    pbias = nc.declare_dram_parameter("pbias", [2, P, H, N], BF16, isOutput=False)
    maskt = nc.declare_dram_parameter("maskt", [2, P, NLOC], F32, isOutput=False)
    w_qt = nc.declare_dram_parameter("w_qt", [CQ, P], BF16, isOutput=False)
    w_kt = nc.declare_dram_parameter("w_kt", [CQ, P], BF16, isOutput=False)
    w_vt = nc.declare_dram_parameter("w_vt", [CQ, P], BF16, isOutput=False)
    w_gt = nc.declare_dram_parameter("w_gt", [CQ, P], BF16, isOutput=False)
    w_ot = nc.declare_dram_parameter("w_ot", [P, CQ], BF16, isOutput=False)
    out = nc.declare_dram_parameter("out", [ROWS, CQ], F32, isOutput=True)

    with tile.TileContext(nc) as tc, ExitStack() as ctx:
        const = ctx.enter_context(tc.tile_pool(name="const", bufs=1))

        NCHUNK = 1
        CROWS = ROWS // NCHUNK
        NSUB = 8
        SROWS = CROWS // NSUB          # 1024 rows per load subtile
        qx_sb = [[const.tile([P, SROWS], BF16, name=f"qx_sb{i}_{s}")
                  for s in range(NSUB)] for i in range(NCHUNK)]
        kx_sb = [[const.tile([P, SROWS], BF16, name=f"kx_sb{i}_{s}")
                  for s in range(NSUB)] for i in range(NCHUNK)]
        for i in range(NCHUNK):
            for s in range(NSUB):
                off = i * CROWS + s * SROWS
                # split the two input streams across independent DMA
                # queues (sync + gpsimd) so they transfer in parallel
                nc.sync.dma_start(out=qx_sb[i][s][:],
                                  in_=q_xt[:, off:off + SROWS])
                nc.gpsimd.dma_start(out=kx_sb[i][s][:],
                                     in_=kv_xt[:, off:off + SROWS])
        wq_sb = const.tile([CQ, P], BF16)
        nc.sync.dma_start(out=wq_sb[:], in_=w_qt[:])
        wk_sb = const.tile([CQ, P], BF16)
        nc.sync.dma_start(out=wk_sb[:], in_=w_kt[:])
        wv_sb = const.tile([CQ, P], BF16)
        nc.sync.dma_start(out=wv_sb[:], in_=w_vt[:])
        wg_sb = const.tile([CQ, P], BF16)
        nc.sync.dma_start(out=wg_sb[:], in_=w_gt[:])
        wo_sb = const.tile([P, CQ], BF16)
        nc.sync.dma_start(out=wo_sb[:], in_=w_ot[:])

        mask_sb = const.tile([P, 2, NLOC], F32)
        nc.sync.dma_start(out=mask_sb[:], in_=maskt.rearrange("t p n -> p t n"))

        pb_sb = const.tile([P, 2, H, N], BF16)
        nc.sync.dma_start(out=pb_sb[:], in_=pbias.rearrange("t p h q -> p t h q"))
        expb_sb = const.tile([P, 2, H, N], BF16)
        nc.scalar.activation(out=expb_sb[:], in_=pb_sb[:], func=AF.Exp)

        # 2.0 (not 1.0): bakes the 0.5 of sigmoid(x)=0.5*(1+tanh(x/2)) into
        # the softmax denominator so inv = 0.5/sum
        ones_sb = const.tile([P, C], BF16)
        nc.vector.memset(ones_sb[:], 2.0)

        qhat = [const.tile([P, CROWS], BF16, name=f"qhat{i}")
                for i in range(NCHUNK)]
        khat = [const.tile([P, CROWS], BF16, name=f"khat{i}")
                for i in range(NCHUNK)]
        ghat = [const.tile([P, CROWS], BF16, name=f"ghat{i}")
                for i in range(NCHUNK)]
        vnat = [const.tile([P, CROWS // P, P], BF16, name=f"vnat{i}")
                for i in range(NCHUNK)]

        # ------- interleaved: projections chunk i, then attention rows -------
        CHUNK = 512
        with tc.tile_pool(name="scps", bufs=2, space="PSUM") as sp, \
             tc.tile_pool(name="accps", bufs=4, space="PSUM") as acc, \
             tc.tile_pool(name="work", bufs=4) as wrk, \
             tc.tile_pool(name="expp", bufs=6) as expp:

            def emit_proj_chunk(i):
                for s in range(NSUB):
                    for ch in range(SROWS // CHUNK):
                        sl = slice(ch * CHUNK, (ch + 1) * CHUNK)
                        gsl = slice(s * SROWS + ch * CHUNK,
                                    s * SROWS + (ch + 1) * CHUNK)
                        ps_q = acc.tile([P, CHUNK], F32, tag="acc",
                                        name=f"ps_q{i}_{s}_{ch}")
                        nc.tensor.matmul(ps_q[:], lhsT=wq_sb[:],
                                         rhs=qx_sb[i][s][:, sl],
                                         start=True, stop=True)
                        nc.vector.tensor_copy(qhat[i][:, gsl], ps_q[:])

                        ps_k = acc.tile([P, CHUNK], F32, tag="acc",
                                        name=f"ps_k{i}_{s}_{ch}")
                        nc.tensor.matmul(ps_k[:], lhsT=wk_sb[:],
                                         rhs=kx_sb[i][s][:, sl],
                                         start=True, stop=True)
                        nc.vector.tensor_copy(khat[i][:, gsl], ps_k[:])

                    for grp in range(SROWS // (4 * P)):
                        ps_v = acc.tile([P, 4, P], F32, tag="acc",
                                        name=f"ps_v{i}_{s}_{grp}")
                        for j in range(4):
                            rt = grp * 4 + j
                            nc.tensor.matmul(
                                ps_v[:, j, :],
                                lhsT=kx_sb[i][s][:, rt * P:(rt + 1) * P],
                                rhs=wv_sb[:],
                                start=(j == 0), stop=(j == 3))
                        goff = (s * SROWS) // P + grp * 4
                        # ACT-side copy: Copy shares the exp/tanh table set
                        nc.scalar.copy(vnat[i][:, goff:goff + 4, :], ps_v[:])

            expT = {}
            expRs = {}

            def emit_scores(n):
                q0 = n * N
                cidx = q0 // CROWS
                coff = cidx * CROWS
                expR = expp.tile([P, 2, H, N], BF16, tag="expR",
                                 name=f"expR{n}")
                expT[n] = expp.tile([P, 2, H, N], BF16, tag="expT",
                                    name=f"expT{n}")
                for t in range(2):
                    k0 = q0 + t * P
                    for hp in range(2):
                        scp = sp.tile([P, 2, 512], F32, tag="sc",
                                      name=f"sc_{n}_{t}_{hp}")
                        for hi in range(2):
                            h = 2 * hp + hi
                            nc.tensor.matmul(
                                scp[:, hi, :N],
                                lhsT=khat[cidx][32 * h:32 * h + 32,
                                                k0 - coff:k0 - coff + P],
                                rhs=qhat[cidx][32 * h:32 * h + 32,
                                               q0 - coff:q0 - coff + N],
                                start=True, stop=True,
                                tile_position=(32 * h, 0),
                            )
                        # exp(s + mask): mask rides the per-partition
                        # activation bias port.
                        if use_mask:
                            nc.scalar.activation(
                                out=expR[:, t, 2 * hp:2 * hp + 2, :],
                                in_=scp[:, :, :N], func=AF.Exp,
                                bias=mask_sb[:, t, n:n + 1])
                        else:
                            nc.scalar.activation(
                                out=expR[:, t, 2 * hp:2 * hp + 2, :],
                                in_=scp[:, :, :N], func=AF.Exp)
                    # * exp(pair_bias): only t0's multiply here; t1's is
                    # deferred past ovsums(n-1) so the ready norm work isn't
                    # stuck behind it in the in-order DVE queue
                    if t == 0:
                        nc.vector.tensor_mul(out=expT[n][:, t],
                                             in0=expR[:, t],
                                             in1=expb_sb[:, t])
                    else:
                        expRs[n] = expR

            def emit_mul_t1(n):
                expR = expRs.pop(n)
                nc.vector.tensor_mul(out=expT[n][:, 1], in0=expR[:, 1],
                                     in1=expb_sb[:, 1])

            def emit_gproj(s):
                for ch in range(SROWS // CHUNK):
                    sl = slice(s * SROWS + ch * CHUNK,
                               s * SROWS + (ch + 1) * CHUNK)
                    ps_g = acc.tile([P, CHUNK], F32, tag="acc",
                                    name=f"ps_g{s}_{ch}")
                    nc.tensor.matmul(ps_g[:], lhsT=wg_sb[:],
                                     rhs=qx_sb[0][s][:, sl - 0 if False else slice(ch * CHUNK, (ch + 1) * CHUNK)],
                                     start=True, stop=True)
                    # sigmoid(x)=0.5*(1+tanh(x/2)); same ACT table as exp
                    nc.scalar.activation(out=ghat[0][:, sl], in_=ps_g[:],
                                         func=AF.Tanh, scale=0.5)

            gated = {}

            def emit_ovsums(n):
                q0 = n * N
                cidx = q0 // CROWS
                coff = cidx * CROWS
                et = expT.pop(n)
                sums_ps = acc.tile([P, N], F32, tag="acc", name=f"sums_ps{n}")
                o_ps = acc.tile([P, N], F32, tag="acc", name=f"o_ps{n}")
                for t in range(2):
                    for h in range(H):
                        nc.tensor.matmul(
                            sums_ps[32 * h:32 * h + 32, :],
                            lhsT=ones_sb[:],
                            rhs=et[:, t, h, :],
                            start=(t == 0), stop=(t == 1),
                            tile_position=(0, 32 * h),
                            skip_group_check=True,
                        )
                    for h in range(H):
                        nc.tensor.matmul(
                            o_ps[32 * h:32 * h + 32, :],
                            lhsT=vnat[cidx][:, (q0 - coff) // P + t,
                                            32 * h:32 * h + 32],
                            rhs=et[:, t, h, :],
                            start=(t == 0), stop=(t == 1),
                            tile_position=(0, 32 * h),
                            skip_group_check=True,
                        )

                inv = wrk.tile([P, N], F32, tag="inv", name=f"inv{n}")
                nc.vector.reciprocal_approx_fast(out=inv[:], in_=sums_ps[:])
                onrm = wrk.tile([P, N], BF16, tag="onrm", name=f"onrm{n}")
                nc.vector.tensor_mul(out=onrm[:], in0=o_ps[:], in1=inv[:])
                gated[n] = wrk.tile([P, N], BF16, tag="gated", name=f"gated{n}")
                nc.vector.scalar_tensor_tensor(
                    out=gated[n][:],
                    in0=ghat[cidx][:, q0 - coff:q0 - coff + N],
                    scalar=1.0,
                    in1=onrm[:],
                    op0=mybir.AluOpType.add,
                    op1=mybir.AluOpType.mult)

            def emit_wo(n):
                q0 = n * N
                gt = gated.pop(n)
                wo_ps = acc.tile([P, 2, 256], F32, tag="acc", name=f"wo_ps{n}")
                for qt in range(2):
                    nc.tensor.matmul(wo_ps[:, qt, :CQ],
                                     lhsT=gt[:, qt * P:(qt + 1) * P],
                                     rhs=wo_sb[:],
                                     start=(qt == 0), stop=(qt == 1))
                osb = wrk.tile([P, 2, CQ], F32, tag="osb", name=f"osb{n}")
                nc.vector.tensor_copy(osb[:], wo_ps[:, :, :CQ])
                nc.sync.dma_start(
                    out=out[q0:q0 + N, :].rearrange("(t p) c -> p t c", p=P),
                    in_=osb[:])

            emit_proj_chunk(0)
            emit_gproj(0)
            for n in range(NLOC):
                emit_scores(n)
                if n - 1 >= 0:
                    emit_ovsums(n - 1)
                emit_mul_t1(n)
                if n - 2 >= 0:
                    emit_wo(n - 2)
                if n >= 2 and (n + 2) % 4 == 0 and (n + 2) // 4 < NSUB:
                    emit_gproj((n + 2) // 4)
            emit_ovsums(NLOC - 1)
            emit_wo(NLOC - 2)
            emit_wo(NLOC - 1)

    nc.compile()
    return nc


_CACHE: dict = {}


def _get_nc(use_mask: bool = False) -> bass.Bass:
    key = ("nc", use_mask)
    if key not in _CACHE:
        _CACHE[key] = build_nc(use_mask=use_mask)
    return _CACHE[key]


def make_in_maps(q_x, kv_x, mask_bias, pair_bias, w_q, w_k, w_v, w_g, w_o):
    qf = np.asarray(q_x, dtype=np.float32).reshape(NCORES, ROWS, CQ)
    kf = np.asarray(kv_x, dtype=np.float32).reshape(NCORES, ROWS, CQ)
    mb = np.asarray(mask_bias, dtype=np.float32).reshape(N, N)      # [n, k]
    pb = np.asarray(pair_bias, dtype=np.float32).reshape(H, N, N)   # [h, q, k]

    pbT = np.transpose(pb, (2, 0, 1))                 # [k, h, q]
    pb_dev = np.ascontiguousarray(pbT.reshape(2, P, H, N)).astype(NP_BF16)

    wqt = np.ascontiguousarray((w_q / np.sqrt(C)).T).astype(NP_BF16)
    wkt = np.ascontiguousarray(np.asarray(w_k).T).astype(NP_BF16)
    wvt = np.ascontiguousarray(np.asarray(w_v).T).astype(NP_BF16)
    wgt = np.ascontiguousarray(np.asarray(w_g).T).astype(NP_BF16)
    wot = np.ascontiguousarray(np.asarray(w_o).T).astype(NP_BF16)   # [hc, cq]

    in_maps = []
    for c in range(NCORES):
        m = mb[c * NLOC:(c + 1) * NLOC]               # [nloc, k]
        mT = np.ascontiguousarray(
            np.transpose(m.reshape(NLOC, 2, P), (1, 2, 0))).astype(np.float32)
        in_maps.append({
            "q_xt": np.ascontiguousarray(qf[c].T).astype(NP_BF16),
            "kv_xt": np.ascontiguousarray(kf[c].T).astype(NP_BF16),
            "pbias": pb_dev,
            "maskt": mT,
            "w_qt": wqt, "w_kt": wkt, "w_vt": wvt, "w_gt": wgt, "w_ot": wot,
        })
    return in_maps


def gather_out(res):
    outs = [np.asarray(res.results[c]["out"], dtype=np.float32)
            for c in range(NCORES)]
    return np.concatenate(outs, axis=0).reshape(B, N, N, CQ)


def kernel(q_x, kv_x, mask_bias, pair_bias, w_q, w_k, w_v, w_g, w_o):
    from concourse.bass_utils import run_bass_kernel_spmd

    nc = _get_nc(use_mask=bool(np.any(np.asarray(mask_bias))))
    in_maps = make_in_maps(q_x, kv_x, mask_bias, pair_bias,
                           w_q, w_k, w_v, w_g, w_o)
    res = run_bass_kernel_spmd(nc, in_maps, list(range(NCORES)))
    return gather_out(res)

